# revision 4
# baseline (speedup 1.0000x reference)
"""AdaAggLayer Trainium2 kernel — 1D Winograd F(2,3) along W.

Data-parallel over batch: 8 NeuronCores x 4 samples each.

The 3x3 conv is decomposed as Winograd F(2,3) along the width axis only:
per (kh row, output-column pair) the 3 kw taps become 4 Winograd taps, so
the PE does 12 tap-matmuls per output instead of 18 shifted matmuls — a
1.5x cut in TensorE cycles (the roofline engine). The input taps
  t0 = xe[j]-xe[j+1], t1 = xo[j]+xe[j+1], t2 = xe[j+1]-xo[j],
  t3 = xo[j]-xo[j+1]
are built from host-split even/odd column planes (pure layout) as
contiguous tensor_tensor adds on DVE (never GpSimd: concurrent Pool
tensor ops degrade DVE ~6x via SBUF contention).

Aggregation happens in RAW KW SPACE (4608 elem/partition/sample instead
of 6144 in tap space — a 25% cut in the dominant vector-engine cost):
the integer-tap variant G' = [w0, w0+w1+w2, w0-w1+w2, w2] is used, whose
taps 0/3 are zero-cost views of the aggregated kw planes; taps 1/2 are
built per-sample with 3 small DVE adds (u = w0+w2; t1 = u+w1;
t2 = u-w1). Expert muls 1-4 run as ACT scaled copies (ACT has slack;
DVE is critical); expert 0 multiplies directly into the kw tile as a
4x-mode tensor_scalar (fully contiguous APs — strided APs drop
tensor_scalar to 2x). The weight DMA is chunked per (ot, expert) and
ordered xe0 / wt-ot0 / wt-ot1 / xe1 so sample 0's ot1 aggregation never
gates on the x DMA of sample 1. Aggregation for sample b+1 is hooked
after each ot of sample b's conv so the PE never waits on weights at
sample boundaries. Conv matmuls run tap-outer [0,3,1,2] so the kw-view
taps start before the tap-1/2 builds land.

PSUM taps 1,2 accumulate in one two-bank tile ([2,16,32] f32: each slot
exactly one bank) and evacuate in a SINGLE ACT op with scale=1/2 (the
missing 1/2 of the integer weight taps). The bias rides the tap-0/3
evacs instead (+b on ev0, -b on ev3: y_even = (e0+b) + e1/2 + e2/2,
y_odd = e1/2 - e2/2 - (e3-b) — algebraically identical). The inverse
transform y_even = ev0+ev1+ev2, y_odd = ev1-ev2-ev3 runs on DVE in bf16
straight off the evacuations. Output is stored as [h, parity, w'] and
interleaved on the host (pure layout).
"""

import contextlib
import importlib.util
import sys
import types

sys.path.insert(0, "/opt/trn_rl_repo")

import numpy as np
import ml_dtypes

import concourse.bass as bass
import concourse.mybir as mybir
import concourse.tile as tile
from concourse import bacc
from concourse.bass_utils import run_bass_kernel_spmd

N_CORES = 8
B, I, O, E, HID = 32, 256, 256, 5, 65
H = W = 56
HP = H + 2  # zero-padded spatial rows
WE = 29  # even/odd column plane width (padded 58 cols split)
WT = 28  # winograd output-pair columns
KH = 3
KW = 3
TAP = 4
NBLK = 4  # row blocks of 14 output rows
RB = 14
BF16 = mybir.dt.bfloat16
F32 = mybir.dt.float32

_NC_CACHE = None


def _install_ntff_hook():
    """Register the axon NTFF profiling hook (the image's antenv lacks it)."""
    if "antenv.axon_hooks" in sys.modules:
        return
    try:
        spec = importlib.util.spec_from_file_location(
            "trn_boot", "/root/.axon_site/trn_agent_boot/trn_boot.py"
        )
        tb = importlib.util.module_from_spec(spec)
        spec.loader.exec_module(tb)
        hook = tb._ntff_profile_via_ctypes("/opt/axon/libaxon_pjrt.so")
    except Exception:
        hook = None
    mod = types.ModuleType("antenv.axon_hooks")
    mod.get_axon_ntff_profile_hook = lambda: hook
    sys.modules["antenv.axon_hooks"] = mod


def _emit(nc, tc, ctx):
    xe_d = nc.dram_tensor("xe", [4, I, HP, WE], BF16, kind="ExternalInput")
    xo_d = nc.dram_tensor("xo", [4, I, HP, WE], BF16, kind="ExternalInput")
    # raw aligned expert weights, chunk-ordered for per-(ot,e) DMA:
    # [ot, e, p=i%128, kh, kw, ih, o128]
    w_d = nc.dram_tensor("wt", [2, E, 128, KH * KW * 2 * 128], BF16, kind="ExternalInput")
    # packed small constants: [p, 0:130]=w1T (2 ih), [0:66, 130:135]=w2Ta,
    # [0:5, 135:391]=bias. One DMA: each dma_start costs ~650ns of serial
    # trigger issue on the sync sequencer.
    misc_d = nc.dram_tensor("misc", [128, 391], F32, kind="ExternalInput")
    out_d = nc.dram_tensor("out", [4, O, H, 2, WT], BF16, kind="ExternalOutput")

    const = ctx.enter_context(tc.tile_pool(name="const", bufs=1))
    xpl = ctx.enter_context(tc.tile_pool(name="xpl", bufs=1))
    xtp = ctx.enter_context(tc.tile_pool(name="xt", bufs=1))
    aggp = ctx.enter_context(tc.tile_pool(name="agg", bufs=1))
    tmpp = ctx.enter_context(tc.tile_pool(name="tmp", bufs=2))
    evp = ctx.enter_context(tc.tile_pool(name="ev", bufs=1))
    ytp = ctx.enter_context(tc.tile_pool(name="yt", bufs=3))
    stagep = ctx.enter_context(tc.tile_pool(name="stage", bufs=4))
    s_psum = ctx.enter_context(tc.tile_pool(name="sps", bufs=1, space="PSUM"))
    a_psum = ctx.enter_context(tc.tile_pool(name="aps", bufs=3, space="PSUM"))
    b_psum = ctx.enter_context(tc.tile_pool(name="bps", bufs=2, space="PSUM"))

    # ---- SBUF constants ----
    # raw aligned expert weights: [p, ot, e, kh, kw, ih, o]
    walT = const.tile([128, 2, E, KH, KW, 2, 128], BF16)
    pscr = const.tile([128, 2, 29, WE], BF16)  # pooling tree scratch
    misc_sb = const.tile([128, 391], F32)
    w1_v = lambda ih: misc_sb[:, ih * HID : (ih + 1) * HID]
    w2_sb = misc_sb[: HID + 1, 130:135]
    bias_sb = misc_sb[:E, 135:391]
    ones_sb = const.tile([1, 128], F32)
    pooledT = const.tile([128, 2, 4], F32)
    h_sb = const.tile([HID + 1, 4], F32)  # row HID is constant 1.0
    att_sb = const.tile([E, 4], F32)
    att_row = const.tile([1, 4 * E], F32)
    att_bc = const.tile([128, 4, E], F32)
    aggb_sb = const.tile([128, 2, 4], F32)
    aggbn_sb = const.tile([128, 2, 4], F32)  # negated bias (rides ev3)

    xe_sb = {}
    xt_sb = {}

    def dma_xplanes(b):
        for ih in range(2):
            te = xpl.tile([128, HP, WE], BF16, tag=f"xe_{ih}", bufs=2, name=f"xe{b}_{ih}")
            to = xpl.tile([128, HP, WE], BF16, tag=f"xo_{ih}", bufs=2, name=f"xo{b}_{ih}")
            nc.sync.dma_start(out=te[:, :, :], in_=xe_d[b, ih * 128 : (ih + 1) * 128])
            nc.sync.dma_start(out=to[:, :, :], in_=xo_d[b, ih * 128 : (ih + 1) * 128])
            xe_sb[(b, ih)] = (te, to)

    def build_xt_taps(b, ih, taps):
        # winograd input taps as contiguous tensor_tensor ops (DVE 2x).
        # NOTE: never place these on gpsimd — concurrent Pool tensor ops
        # degrade DVE throughput ~6x (SBUF contention).
        if (b, ih) in xt_sb:
            t = xt_sb[(b, ih)]
        else:
            t = xtp.tile(
                [128, TAP, HP, WT], BF16, tag=f"xt_{ih}", bufs=2, name=f"xt{b}_{ih}"
            )
            xt_sb[(b, ih)] = t
        xe, xo = xe_sb[(b, ih)]
        for tap in taps:
            if tap == 0:
                nc.vector.tensor_sub(out=t[:, 0], in0=xe[:, :, 0:28], in1=xe[:, :, 1:29])
            elif tap == 1:
                nc.vector.tensor_add(out=t[:, 1], in0=xo[:, :, 0:28], in1=xe[:, :, 1:29])
            elif tap == 2:
                nc.vector.tensor_sub(out=t[:, 2], in0=xe[:, :, 1:29], in1=xo[:, :, 0:28])
            else:
                nc.vector.tensor_sub(out=t[:, 3], in0=xo[:, :, 0:28], in1=xo[:, :, 1:29])

    def _pool_tree(b, ih, ncols):
        s = pscr[:, ih]
        nc.vector.tensor_add(
            out=s[:, 0:14, 0:ncols], in0=s[:, 0:14, 0:ncols], in1=s[:, 15:29, 0:ncols]
        )
        nc.vector.tensor_add(
            out=s[:, 0:7, 0:ncols], in0=s[:, 0:7, 0:ncols], in1=s[:, 7:14, 0:ncols]
        )
        nc.vector.tensor_add(
            out=s[:, 0:1, 0:ncols], in0=s[:, 0:1, 0:ncols], in1=s[:, 14:15, 0:ncols]
        )
        nc.vector.reduce_sum(
            out=pooledT[:, ih, b : b + 1],
            in_=s[:, 0:7, 0:ncols],
            axis=mybir.AxisListType.XY,
        )

    def pool_half(b, ih):
        # pooled sum from winograd tap 1: sum_j (xo[j]+xe[j+1]) telescopes to
        # the full (zero-padded) row sum, so the tap plane doubles as the
        # pooling input. Pairwise row-fold tree in 2x bf16.
        s = pscr[:, ih]
        t1 = xt_sb[(b, ih)][:, 1]
        nc.vector.tensor_add(
            out=s[:, 0:29, 0:WT], in0=t1[:, 0:29, :], in1=t1[:, 29:58, :]
        )
        _pool_tree(b, ih, WT)

    def attention_tail(b0, nb, hp):
        nc.scalar.activation(
            h_sb[:HID, b0 : b0 + nb], hp[:, :nb], mybir.ActivationFunctionType.Relu
        )
        ap = s_psum.tile([E, 4], F32, tag="sps", name="ap")
        nc.tensor.matmul(ap[:, :nb], lhsT=w2_sb[:, :], rhs=h_sb[:, b0 : b0 + nb])
        nc.scalar.activation(
            att_sb[:, b0 : b0 + nb], ap[:, :nb], mybir.ActivationFunctionType.Sigmoid
        )
        rp = s_psum.tile([1, 4 * E], F32, tag="sps", name="rp")
        for j in range(nb):
            nc.tensor.matmul(
                rp[0:1, j * E : (j + 1) * E],
                lhsT=h_sb[:, b0 + j : b0 + j + 1],
                rhs=w2_sb[:, :],
            )
        nc.scalar.activation(
            att_row[0:1, b0 * E : (b0 + nb) * E],
            rp[0:1, : nb * E],
            mybir.ActivationFunctionType.Sigmoid,
        )
        bp = s_psum.tile([128, 4 * E], F32, tag="sps", name="bp")
        nc.tensor.matmul(
            bp[:, : nb * E],
            lhsT=ones_sb[0:1, :],
            rhs=att_row[0:1, b0 * E : (b0 + nb) * E],
        )
        nc.scalar.activation(
            att_bc[:, b0 : b0 + nb, :], bp[:, : nb * E],
            mybir.ActivationFunctionType.Identity,
        )
        for ot in range(2):
            gp = s_psum.tile([128, 4], F32, tag="sps", name="gp")
            nc.tensor.matmul(
                gp[:, :nb],
                lhsT=bias_sb[:, ot * 128 : (ot + 1) * 128],
                rhs=att_sb[:, b0 : b0 + nb],
            )
            nc.scalar.activation(
                aggb_sb[:, ot, b0 : b0 + nb], gp[:, :nb],
                mybir.ActivationFunctionType.Identity,
            )
            nc.scalar.activation(
                aggbn_sb[:, ot, b0 : b0 + nb], gp[:, :nb],
                mybir.ActivationFunctionType.Identity, scale=-1.0,
            )

    def attention_n(b0, nb):
        hp = s_psum.tile([HID, 4], F32, tag="sps", name="hp")
        for ih in range(2):
            nc.tensor.matmul(
                hp[:, :nb],
                lhsT=w1_v(ih),
                rhs=pooledT[:, ih, b0 : b0 + nb],
                start=(ih == 0),
                stop=(ih == 1),
            )
        attention_tail(b0, nb, hp)

    # per-sample kw-space aggregation: ACT scaled copies for experts 1-4,
    # DVE 4x tensor_scalar for expert 0 + the 4 adds, then the 2 non-view
    # winograd weight taps with 3 small adds.
    kw_all = {}
    t12_all = {}

    def agg_build(b, ot):
        kw = aggp.tile(
            [128, KH, KW, 2, 128], BF16, tag=f"kw_{ot}", bufs=2, name=f"kw{b}_{ot}"
        )
        tas = {}
        for e in (1, 2, 3, 4):
            ta = tmpp.tile(
                [128, KH, KW, 2, 128], BF16, tag="ta", bufs=4, name=f"ta{e}"
            )
            nc.scalar.activation(
                ta, walT[:, ot, e], mybir.ActivationFunctionType.Identity,
                scale=att_bc[:, b, e : e + 1],
            )
            tas[e] = ta
        nc.vector.tensor_scalar_mul(kw, walT[:, ot, 0], att_bc[:, b, 0:1])
        for e in (1, 2, 3, 4):
            nc.vector.tensor_add(out=kw, in0=kw, in1=tas[e])
        # weight taps 1,2 (integer G'): u = kw0+kw2; t1 = u+kw1; t2 = u-kw1
        t12 = aggp.tile(
            [128, KH, 2, 2, 128], BF16, tag=f"t12_{ot}", bufs=2, name=f"t12{b}_{ot}"
        )
        uu = tmpp.tile([128, KH, 2, 128], BF16, tag="u", bufs=2, name="uu")
        nc.vector.tensor_add(out=uu, in0=kw[:, :, 0], in1=kw[:, :, 2])
        nc.vector.tensor_add(out=t12[:, :, 0], in0=uu, in1=kw[:, :, 1])
        nc.vector.tensor_sub(out=t12[:, :, 1], in0=uu, in1=kw[:, :, 1])
        kw_all[(b, ot)] = kw
        t12_all[(b, ot)] = t12

    def conv_lhsT(b, ot, kh, tap, ih):
        if tap == 0:
            return kw_all[(b, ot)][:, kh, 0, ih, :]
        if tap == 3:
            return kw_all[(b, ot)][:, kh, 2, ih, :]
        return t12_all[(b, ot)][:, kh, tap - 1, ih, :]

    # ---- DMA schedule ----
    dma_xplanes(0)
    nc.sync.dma_start(out=misc_sb[:, :], in_=misc_d[:, :])
    nc.vector.memset(ones_sb[:, :], 1.0)
    nc.vector.memset(h_sb[HID - 1 : HID + 1, :], 1.0)
    for ot in range(2):
        for e in range(E):
            nc.sync.dma_start(out=walT[:, ot, e], in_=w_d[ot, e])
    dma_xplanes(1)

    # preload the sigmoid ACT table off the critical path
    tscr = const.tile([1, 4], F32, name="tscr")
    nc.scalar.activation(
        tscr[0:1, :], ones_sb[0:1, 0:4], mybir.ActivationFunctionType.Sigmoid
    )

    for ih in range(2):
        build_xt_taps(0, ih, [1])
    for ih in range(2):
        pool_half(0, ih)
    attention_n(0, 1)
    for ih in range(2):
        build_xt_taps(0, ih, [0, 2, 3])
    agg_build(0, 0)
    agg_build(0, 1)
    for ih in range(2):
        build_xt_taps(1, ih, [1])
    for ih in range(2):
        pool_half(1, ih)

    # ---- per-sample winograd conv ----
    for b in range(4):
        for ot in range(2):
            for pair in range(2):
                # ev tiles: taps 0/3 single (carry +bias/-bias); taps 1,2
                # share one tile written by a single paired evac per sub
                ev0 = evp.tile([128, 2, RB, WT], BF16, tag="e0", bufs=2, name="ev0")
                ev3 = evp.tile([128, 2, RB, WT], BF16, tag="e3", bufs=2, name="ev3")
                e12 = evp.tile([128, 2, 2, RB, WT], BF16, tag="e12", bufs=2, name="e12")
                for sub in range(2):
                    blk = pair * 2 + sub
                    r0 = blk * RB
                    a0 = a_psum.tile([128, RB, WT], F32, tag="ta03", name="a0")
                    a3 = a_psum.tile([128, RB, WT], F32, tag="ta03", name="a3")
                    # taps 1,2 in one 2-bank tile: slot stride 16*32=512 f32
                    # = exactly one PSUM bank, so each accumulation region
                    # stays in-bank and one ACT op evacuates both.
                    bt = b_psum.tile([128, 2, 16, 32], F32, tag="tb", name="bt")
                    outs = {
                        0: a0[:, :, :],
                        3: a3[:, :, :],
                        1: bt[:, 0, 0:RB, 0:WT],
                        2: bt[:, 1, 0:RB, 0:WT],
                    }
                    # tap-outer: taps 0,3 (kw views, ready first), then 1,2
                    for tap in (0, 3, 1, 2):
                        for kh in range(KH):
                            for ih in range(2):
                                nc.tensor.matmul(
                                    outs[tap],
                                    lhsT=conv_lhsT(b, ot, kh, tap, ih),
                                    rhs=xt_sb[(b, ih)][
                                        :, tap, r0 + kh : r0 + kh + RB, :
                                    ],
                                    start=(kh == 0 and ih == 0),
                                    stop=(kh == KH - 1 and ih == 1),
                                )
                    # evacs: one paired op for taps 1,2 (scale 1/2 of the
                    # integer weight taps); bias rides ev0 (+b) / ev3 (-b)
                    nc.scalar.activation(
                        e12[:, :, sub],
                        bt[:, :, 0:RB, 0:WT],
                        mybir.ActivationFunctionType.Identity,
                        scale=0.5,
                    )
                    nc.scalar.activation(
                        ev0[:, sub],
                        a0[:, :, :],
                        mybir.ActivationFunctionType.Identity,
                        bias=aggb_sb[:, ot, b : b + 1],
                    )
                    nc.scalar.activation(
                        ev3[:, sub],
                        a3[:, :, :],
                        mybir.ActivationFunctionType.Identity,
                        bias=aggbn_sb[:, ot, b : b + 1],
                    )
                # inverse transform on DVE (bf16 2x), both blocks at once,
                # into the parity-split stage
                st = stagep.tile([128, 2, RB, 2, WT], BF16, tag="stage", bufs=3, name="st")
                y0t = ytp.tile([128, 2, RB, WT], BF16, tag="yt", name="y0t")
                nc.vector.tensor_add(out=y0t, in0=ev0, in1=e12[:, 0])
                nc.vector.tensor_add(out=st[:, :, :, 0, :], in0=y0t, in1=e12[:, 1])
                y1t = ytp.tile([128, 2, RB, WT], BF16, tag="yt", name="y1t")
                nc.vector.tensor_sub(out=y1t, in0=e12[:, 0], in1=e12[:, 1])
                nc.vector.tensor_sub(out=st[:, :, :, 1, :], in0=y1t, in1=ev3)
                nc.sync.dma_start(
                    out=out_d[b, ot * 128 : (ot + 1) * 128, pair * 2 * RB : (pair + 1) * 2 * RB, :, :],
                    in_=st[:, :, :, :, :],
                )
            # pipeline hooks: after ot0 the NEXT sample's attention, its
            # ot0 aggregation and its remaining input taps (so its conv
            # never waits at the sample boundary); after ot1 its ot1
            # aggregation, plus tap-1/pooling for the sample after that.
            if ot == 0 and b < 3:
                attention_n(b + 1, 1)
                for ih in range(2):
                    build_xt_taps(b + 1, ih, [0, 2, 3])
                agg_build(b + 1, 0)
                if b + 2 < 4:
                    dma_xplanes(b + 2)
            if ot == 1 and b < 3:
                agg_build(b + 1, 1)
                if b < 2:
                    for ih in range(2):
                        build_xt_taps(b + 2, ih, [1])
                    for ih in range(2):
                        pool_half(b + 2, ih)


def _build():
    nc = bacc.Bacc("TRN2", target_bir_lowering=False, debug=False, num_devices=N_CORES)
    with contextlib.ExitStack() as ctx:
        tc = ctx.enter_context(tile.TileContext(nc))
        _emit(nc, tc, ctx)
    nc.compile()
    return nc


def _get_nc():
    global _NC_CACHE
    if _NC_CACHE is None:
        _NC_CACHE = _build()
    return _NC_CACHE


def _run(trace=False, **inputs):
    BL = 4
    x = np.asarray(inputs["x"], np.float32)
    weight = np.asarray(inputs["weight"], np.float32)
    bias = np.asarray(inputs["bias"], np.float32)
    align = np.asarray(inputs["align"], np.float32)
    w1 = np.asarray(inputs["attn_w1"], np.float32)
    w2 = np.asarray(inputs["attn_w2"], np.float32)
    b2 = np.asarray(inputs["attn_b2"], np.float32)

    xp = np.zeros((B, I, HP, HP), dtype=ml_dtypes.bfloat16)
    xp[:, :, 1 : 1 + H, 1 : 1 + W] = x
    xe = np.ascontiguousarray(xp[:, :, :, 0::2])
    xo = np.ascontiguousarray(xp[:, :, :, 1::2])

    # host: fold align (weight-only reparam), then lay out raw kw-space
    # chunk-ordered: wt[ot, e, p, kh, kw, ih, o]
    #   = w_al[e, ot*128+o, ih*128+p, kh, kw]
    w_al = np.einsum("eno,eok->enk", align, weight.reshape(E, O, I * 9)).reshape(
        E, 2, 128, 2, 128, KH, KW
    )  # [e, ot, o, ih, p, kh, kw]
    wt = np.ascontiguousarray(w_al.transpose(1, 0, 4, 5, 6, 3, 2)).astype(
        ml_dtypes.bfloat16
    )  # [ot, e, p, kh, kw, ih, o]
    wt = wt.reshape(2, E, 128, KH * KW * 2 * 128)

    w1T = (w1 / float(H * W)).T.reshape(2, 128, HID)
    w2Ta = np.concatenate([w2.T, b2.reshape(1, E)], axis=0)
    misc = np.zeros((128, 391), np.float32)
    misc[:, 0:HID] = w1T[0]
    misc[:, HID : 2 * HID] = w1T[1]
    misc[: HID + 1, 130:135] = w2Ta
    misc[:E, 135:391] = bias

    nc = _get_nc()
    in_maps = []
    for c in range(N_CORES):
        in_maps.append(
            {
                "xe": xe[c * BL : (c + 1) * BL],
                "xo": xo[c * BL : (c + 1) * BL],
                "wt": wt,
                "misc": misc,
            }
        )
    if trace:
        _install_ntff_hook()
    res = run_bass_kernel_spmd(
        nc, in_maps, core_ids=list(range(N_CORES)), trace=trace
    )
    out = np.concatenate([res.results[c]["out"] for c in range(N_CORES)], axis=0)
    # interleave the parity planes: [B,O,H,2,28] -> [B,O,H,56]
    out = out.transpose(0, 1, 2, 4, 3).reshape(B, O, H, W)
    return out.astype(np.float32), res


def kernel(**inputs):
    out, _ = _run(trace=False, **inputs)
    return out


def kernel_profiled(**inputs):
    out, res = _run(trace=True, **inputs)
    return out, res
